# revision 1
# baseline (speedup 1.0000x reference)
"""Falcon-style MQA attention (71 heads, 1 KV head, RoPE, causal) on 8 TRN2 NeuronCores.

Sharding: tensor-parallel over query heads (9 per core, core 7 has 8 + a zero-pad
head), the single KV head replicated. Per core: QKV projection for its heads
(+KV), RoPE, causal flash-style attention in transposed layout, then a PARTIAL
dense projection over the core's own head rows for all 4544 output columns.
The host sums the 8 partial outputs (contraction-sharded dense = host reduce);
no device collective is needed. All operand transposes are done host-side.

Self-contained: hardcodes all shapes; needs only numpy + ml_dtypes + concourse.
"""

import math
from contextlib import ExitStack

import numpy as np
import ml_dtypes

import concourse.bass as bass
import concourse.mybir as mybir
import concourse.tile as tile
from concourse import bacc
from concourse.bass_utils import run_bass_kernel_spmd

NCORES = 8
N, L, D = 2, 1024, 4544
H, DKV = 71, 64
M = N * L                    # 2048 tokens
DP = 4608                    # D padded to 36*128
KT = DP // 128               # 36 contraction tiles for QKV
HPC = 9                      # head slots per core (core 7: 8 real + 1 zero-pad)
QROWS = HPC * DKV            # 576 attention rows per core
QPAD = 640                   # padded to 5*128 for the dense contraction
RROWS = QROWS + 2 * DKV      # 704 fused rows per core (q + k + v)
RC = 6                       # row-chunks of fusedT (5x128 + 64)
MCH = 256                    # QKV token-chunk width
ROPE_BASE = 10000.0

F32 = mybir.dt.float32
F32R = mybir.dt.float32r
BF16 = mybir.dt.bfloat16


def _build():
    nc = bacc.Bacc("TRN2", target_bir_lowering=False, debug=False, num_devices=NCORES)

    hs_bf = nc.dram_tensor("hs_bf", [DP, M], BF16, kind="ExternalInput")      # hs.T
    wq_bf = nc.dram_tensor("wq_bf", [DP, RROWS], BF16, kind="ExternalInput")  # wq_loc.T
    wd_bf = nc.dram_tensor("wd_bf", [QPAD, D], BF16, kind="ExternalInput")    # wd rows for local heads
    cos2 = nc.dram_tensor("cos2", [128, L], F32, kind="ExternalInput")
    sin2 = nc.dram_tensor("sin2", [128, L], F32, kind="ExternalInput")
    tri_in = nc.dram_tensor("tri", [128, 128], F32, kind="ExternalInput")
    prope2 = nc.dram_tensor("prope2", [128, 128], F32R, kind="ExternalInput")
    ident64 = nc.dram_tensor("ident64", [64, 64], F32R, kind="ExternalInput")
    colones = nc.dram_tensor("colones", [128, 16], F32R, kind="ExternalInput")
    ones1 = nc.dram_tensor("ones1", [1, 64], F32R, kind="ExternalInput")
    out = nc.dram_tensor("out", [M, D], F32, kind="ExternalOutput")

    with tile.TileContext(nc) as tc, ExitStack() as top:
        constp = top.enter_context(tc.tile_pool(name="const", bufs=1))
        workp = top.enter_context(tc.tile_pool(name="work", bufs=2))
        psA = top.enter_context(tc.tile_pool(name="psA", bufs=2, space="PSUM"))
        psB = top.enter_context(tc.tile_pool(name="psB", bufs=2, space="PSUM"))
        psC = top.enter_context(tc.tile_pool(name="psC", bufs=2, space="PSUM"))
        psD = top.enter_context(tc.tile_pool(name="psD", bufs=2, space="PSUM"))
        dramp = top.enter_context(tc.tile_pool(name="dram", bufs=1, space="DRAM"))

        # ---- constants ----
        cosT = constp.tile([128, L], F32)
        sinT = constp.tile([128, L], F32)
        tri = constp.tile([128, 128], F32)
        prope = constp.tile([128, 128], F32R)
        id64 = constp.tile([64, 64], F32R)
        ones_1x64 = constp.tile([1, 64], F32R)
        nc.scalar.dma_start(ones_1x64[:], ones1[:])
        nc.scalar.dma_start(cosT[:], cos2[:])
        nc.scalar.dma_start(sinT[:], sin2[:])
        nc.scalar.dma_start(tri[:], tri_in[:])
        nc.scalar.dma_start(prope[:], prope2[:])
        nc.scalar.dma_start(id64[:], ident64[:])

        fusedp = top.enter_context(tc.tile_pool(name="fused", bufs=1))
        fusedT = fusedp.tile([128, RC, M], F32R)

        with ExitStack() as stageA:
            wqp = stageA.enter_context(tc.tile_pool(name="wq", bufs=1))
            hstp = stageA.enter_context(tc.tile_pool(name="hst", bufs=2))

            wqT = wqp.tile([128, KT, RROWS], BF16)
            wq_r = wq_bf[:].rearrange("(kt p) r -> p kt r", p=128)
            for kt in range(KT):
                nc.scalar.dma_start(wqT[:, kt, :], wq_r[:, kt, :])

            # ---- phase 1: fusedT[r, m] = (hs @ wq_loc.T).T ----
            hs_r = hs_bf[:].rearrange("(kt p) m -> p kt m", p=128)
            for mc in range(M // MCH):
                hsT = hstp.tile([128, KT, MCH], BF16, tag="hsT")
                for kt in range(KT):
                    nc.sync.dma_start(hsT[:, kt, :],
                                      hs_r[:, kt, MCH * mc:MCH * (mc + 1)])
                for rc in range(RC):
                    rp = 128 if rc < 5 else 64
                    ps = psA.tile([128, 512], F32, tag="big")
                    for kt in range(KT):
                        nc.tensor.matmul(
                            ps[:rp, :MCH], wqT[:, kt, 128 * rc:128 * rc + rp],
                            hsT[:, kt, :], start=(kt == 0), stop=(kt == KT - 1))
                    nc.vector.tensor_copy(
                        fusedT[:rp, rc, MCH * mc:MCH * (mc + 1)], ps[:rp, :MCH])

            # ---- phase 2: RoPE in place on q rows and the k row ----
            for n in range(N):
                for rc in range(5):
                    x = fusedT[:, rc, L * n:L * (n + 1)]
                    for hf in range(2):
                        sl = slice(512 * hf, 512 * (hf + 1))
                        pp = psB.tile([128, 512], F32, tag="rope")
                        nc.tensor.matmul(pp[:], prope[:], x[:, sl],
                                         start=True, stop=True)
                        a = workp.tile([128, 512], F32, tag="ropea")
                        b = workp.tile([128, 512], F32, tag="ropeb")
                        nc.vector.tensor_mul(a[:], x[:, sl], cosT[:, sl])
                        nc.vector.tensor_mul(b[:], pp[:], sinT[:, sl])
                        nc.vector.tensor_add(x[:, sl], a[:], b[:])

        # ---- stage B: attention + partial dense ----
        stageB = ExitStack()
        wdp = stageB.enter_context(tc.tile_pool(name="wd", bufs=1))
        attnp = stageB.enter_context(tc.tile_pool(name="attn", bufs=1))
        expp = stageB.enter_context(tc.tile_pool(name="exps", bufs=6))

        wdT2 = wdp.tile([128, QPAD // 128, D], BF16)
        wd_r = wd_bf[:].rearrange("(kt p) c -> p kt c", p=128)
        for kt in range(QPAD // 128):
            nc.scalar.dma_start(wdT2[:, kt, :], wd_r[:, kt, :])

        # kT duplicated into both partition halves so lhsT/rhs base partitions
        # match for every head (matmul requires equal base partitions).
        kT_dup = attnp.tile([128, N, L], F32R)
        for n in range(N):
            nc.scalar.dma_start(kT_dup[0:64, n, :], fusedT[64:128, 4, L * n:L * (n + 1)])
            nc.scalar.dma_start(kT_dup[64:128, n, :], fusedT[64:128, 4, L * n:L * (n + 1)])
        v_nat = attnp.tile([128, N * 8, DKV + 1], F32R)
        nc.scalar.dma_start(v_nat[:, :, DKV:DKV + 1],
                            colones[:].rearrange("p (s o) -> p s o", o=1))
        for n in range(N):
            for jt in range(8):
                tp = psC.tile([128, 64], F32R, tag="av")
                nc.tensor.transpose(
                    tp[:], fusedT[0:64, 5, L * n + 128 * jt:L * n + 128 * (jt + 1)],
                    id64[:])
                nc.vector.tensor_copy(v_nat[:, 8 * n + jt, 0:DKV], tp[:])

        # attention output rows (local heads), padded to 640 for the dense
        # contraction; pad rows zeroed (they multiply garbage otherwise)
        attn_sb = attnp.tile([128, QPAD // 128, M], BF16)
        nc.vector.memset(attn_sb[64:128, 4, :], 0.0)

        def attn_head(n, h, spool, stag):
            """Generator: one attention head, yielding between j-tile units."""
            poff = (64 * h) % 128
            prc = (64 * h) // 128
            kTn = kT_dup[poff:poff + 64, n, :]
            qh = fusedT[poff:poff + 64, prc, L * n:L * (n + 1)]
            for qc in range(2):
                av = psC.tile([65, 512], F32, tag="av")
                njt = 4 * (qc + 1)
                pend = None
                for jt in range(njt):
                    off = max(0, 128 * jt - 512 * qc)
                    sp = spool.tile([128, 512], F32, tag=stag)
                    nc.tensor.matmul(
                        sp[:, 0:512 - off],
                        kTn[:, 128 * jt:128 * (jt + 1)],
                        qh[:, 512 * qc + off:512 * (qc + 1)],
                        start=True, stop=True)
                    et = expp.tile([128, 512], F32R, tag="exp")
                    nc.scalar.activation(
                        et[:, off:512], sp[:, 0:512 - off],
                        mybir.ActivationFunctionType.Exp,
                        scale=1.0 / math.sqrt(DKV))
                    if 128 * jt >= 512 * qc:
                        nc.vector.tensor_mul(
                            et[:, off:off + 128], et[:, off:off + 128], tri[:])
                    if pend is not None:
                        pjt, po, pet = pend
                        nc.tensor.matmul(
                            av[:, po:512], v_nat[:, 8 * n + pjt, :], pet[:, po:512],
                            start=(pjt == 0), stop=False)
                    pend = (jt, off, et)
                    yield
                pjt, po, pet = pend
                nc.tensor.matmul(
                    av[:, po:512], v_nat[:, 8 * n + pjt, :], pet[:, po:512],
                    start=(pjt == 0), stop=True)
                rec = workp.tile([1, 512], F32R, tag="rec")
                with nc.allow_low_precision(reason="softmax denom"):
                    nc.vector.reciprocal(rec[:], av[64:65, :])
                yield
                pr = psD.tile([64, 512], F32, tag="pr")
                nc.tensor.matmul(pr[:], ones_1x64[:], rec[:], start=True, stop=True)
                ob32 = workp.tile([64, 512], F32, tag="ob32")
                nc.vector.tensor_copy(ob32[:], av[0:64, :])
                yield
                nc.vector.tensor_mul(
                    attn_sb[poff:poff + 64, prc, L * n + 512 * qc:L * n + 512 * (qc + 1)],
                    ob32[:], pr[:])
                yield

        def run_heads(n, interleave=None):
            """Drive all heads of batch n with 2 rolling in-flight generators.
            interleave: optional list of thunks to emit spread through the run."""
            pools = [(psA, "big"), (psB, "rope")]
            nxt = 0
            slots = []
            for i in range(2):
                slots.append(attn_head(n, nxt, *pools[i]))
                nxt += 1
            inter = list(interleave or [])
            steps = 0
            while slots:
                for g in list(slots):
                    try:
                        next(g)
                    except StopIteration:
                        slots.remove(g)
                        i = len(slots)
                        if nxt < HPC:
                            slots.append(attn_head(n, nxt, *pools[i % 2]))
                            nxt += 1
                        elif inter:
                            inter.pop(0)()
                steps += 1
            while inter:
                inter.pop(0)()

        CCH = [512] * 8 + [448]          # dense column chunks (sum = 4544)

        def dense_mtile(n, mt):
            col = 0
            for w in CCH:
                pa = psB.tile([128, 512], F32, tag="rope")
                for kt in range(QPAD // 128):
                    nc.tensor.matmul(
                        pa[:, :w], attn_sb[:, kt, L * n + 128 * mt:L * n + 128 * (mt + 1)],
                        wdT2[:, kt, col:col + w],
                        start=(kt == 0), stop=(kt == QPAD // 128 - 1))
                ot = workp.tile([128, 512], F32, tag="ot")
                nc.vector.tensor_copy(ot[:, :w], pa[:, :w])
                nc.scalar.dma_start(
                    out[L * n + 128 * mt:L * n + 128 * (mt + 1), col:col + w],
                    ot[:, :w])
                col += w

        # attention batch 0 (rolling heads), then batch 1 with the batch-0
        # dense m-tiles appended as pipeline drain work
        run_heads(0)
        run_heads(1, interleave=[
            (lambda mt=mt: dense_mtile(0, mt)) for mt in range(8)])
        for mt in range(8):
            dense_mtile(1, mt)
        stageB.close()

    nc.compile()
    return nc


_NC_CACHE = None


def _get_nc():
    global _NC_CACHE
    if _NC_CACHE is None:
        _NC_CACHE = _build()
    return _NC_CACHE


def _host_inputs(hidden_states, w_qkv, w_dense):
    """Build the per-core input maps (transpose + slice + bf16 cast on host)."""
    hs = np.asarray(hidden_states, dtype=np.float32).reshape(M, D)
    w_qkv = np.asarray(w_qkv, dtype=np.float32)
    w_dense = np.asarray(w_dense, dtype=np.float32)
    hs_bf = np.zeros((DP, M), dtype=ml_dtypes.bfloat16)
    hs_bf[:D, :] = np.ascontiguousarray(hs.T).astype(ml_dtypes.bfloat16)

    # RoPE tables, transposed to [dkv, l], duplicated on partitions 0-63 / 64-127
    inv_freq = 1.0 / (ROPE_BASE ** (np.arange(0, DKV, 2, dtype=np.float32) / DKV))
    t = np.arange(L, dtype=np.float32)
    freqs = np.outer(t, inv_freq)
    emb = np.concatenate([freqs, freqs], axis=-1)        # [L, DKV]
    cosT = np.cos(emb).T.astype(np.float32)              # [DKV, L]
    sinT = np.sin(emb).T.astype(np.float32)
    cos2 = np.concatenate([cosT, cosT], axis=0)          # [128, L]
    sin2 = np.concatenate([sinT, sinT], axis=0)

    # tri[j, q] = 1 if j <= q (within-tile causal mask)
    tri = (np.arange(128)[:, None] <= np.arange(128)[None, :]).astype(np.float32)

    # RoPE rotation: (P x)[d] = -x[d+32] (d<32), x[d-32] (d>=32); lhsT = P.T, 2 blocks
    P1 = np.zeros((DKV, DKV), dtype=np.float32)
    for d in range(32):
        P1[d, d + 32] = -1.0
        P1[d + 32, d] = 1.0
    PT = P1.T
    prope2 = np.zeros((128, 128), dtype=np.float32)
    prope2[:64, :64] = PT
    prope2[64:, 64:] = PT

    ident64 = np.eye(64, dtype=np.float32)

    kv_bf = w_qkv[H * DKV:, :].T.astype(ml_dtypes.bfloat16)   # [D, 128]
    in_maps = []
    for c in range(NCORES):
        h0 = HPC * c
        nh = min(HPC, H - h0)
        wq_loc = np.zeros((DP, RROWS), dtype=ml_dtypes.bfloat16)
        wq_loc[:D, :nh * DKV] = w_qkv[h0 * DKV:(h0 + nh) * DKV, :].T.astype(
            ml_dtypes.bfloat16)
        wq_loc[:D, QROWS:] = kv_bf

        # dense weight rows for this core's heads: w_dense columns
        # [64*h0 : 64*(h0+nh)) transposed, zero-padded to QPAD rows
        wd_loc = np.zeros((QPAD, D), dtype=ml_dtypes.bfloat16)
        wd_loc[:nh * DKV, :] = w_dense[:, DKV * h0:DKV * (h0 + nh)].T.astype(
            ml_dtypes.bfloat16)

        in_maps.append({
            "hs_bf": hs_bf,
            "wq_bf": wq_loc,
            "wd_bf": wd_loc,
            "cos2": cos2,
            "sin2": sin2,
            "tri": tri,
            "prope2": prope2,
            "ident64": ident64,
            "colones": np.ones((128, 16), dtype=np.float32),
            "ones1": np.ones((1, 64), dtype=np.float32),
        })
    return in_maps


def kernel(hidden_states, w_qkv, w_dense, _trace=False, _trace_kwargs=None):
    nc = _get_nc()
    in_maps = _host_inputs(hidden_states, w_qkv, w_dense)
    kw = {}
    if _trace:
        kw = dict(trace=True, **(_trace_kwargs or {}))
    res = run_bass_kernel_spmd(nc, in_maps, list(range(NCORES)), **kw)
    full = res.results[0]["out"].astype(np.float32)
    for c in range(1, NCORES):
        full += res.results[c]["out"]
    kernel._last_exec_time_ns = res.exec_time_ns
    return full.reshape(N, L, D).astype(np.float32)



# revision 23
# speedup vs baseline: 1.5597x; 1.5597x over previous
"""Falcon-style MQA attention (71 heads, 1 KV head, RoPE, causal) on 8 TRN2 NeuronCores.

Sharding: tensor-parallel over query heads (9 slots per core, core 7 has 8 real
+ 1 zero-pad), single KV head replicated. Per core: QKV projection for its
heads (+KV), RoPE, causal attention in transposed [k, q] layout, then a PARTIAL
dense projection over the core's own head rows for all 4544 output columns.
The host sums the 8 bf16 partial outputs (contraction-sharded dense).

v2: software-pipelined to keep the tensor engine continuously busy (HAM warm):
 - QKV+RoPE pipelined per 256-token chunk
 - remaining QKV / dense GEMMs interleaved as PE filler inside attention
 - reciprocal_approx_fast for softmax denominators (was 3.3us/call reciprocal)
 - bf16 moving operands on all hot matmuls; elementwise spread over Act/DVE/GpSimd
 - coarse 3D DMAs; bf16 output partials summed on host
 - wq/hs SBUF region reused for wd (staged pools)

Self-contained: hardcodes all shapes; needs only numpy + ml_dtypes + concourse.
"""

import math
from contextlib import ExitStack

import numpy as np
import ml_dtypes

import concourse.bass as bass
import concourse.mybir as mybir
import concourse.tile as tile
from concourse import bacc
from concourse.bass_utils import run_bass_kernel_spmd

NCORES = 8
N, L, D = 2, 1024, 4544
H, DKV = 71, 64
M = N * L                    # 2048 tokens
DP = 4608                    # D padded to 36*128
KT = DP // 128               # 36 contraction tiles for QKV
HPC = 9                      # head slots per core (core 7: 8 real + 1 zero-pad)
QROWS = HPC * DKV            # 576 q rows per core
QPAD = 640                   # padded to 5*128 for the dense contraction
RROWS = QROWS + 2 * DKV      # 704 fused rows per core (q + k + v)
RC = 6                       # row-chunks of the QKV output (5x128 + 64)
MCH = 256                    # QKV token-chunk width (8 chunks)
NMC = M // MCH
ROPE_BASE = 10000.0

F32 = mybir.dt.float32
F32R = mybir.dt.float32r
BF16 = mybir.dt.bfloat16
COPYF = mybir.ActivationFunctionType.Copy
EXPF = mybir.ActivationFunctionType.Exp

_DEBUG_DUMPS = False


def _build():
    nc = bacc.Bacc("TRN2", target_bir_lowering=False, debug=False, num_devices=NCORES)

    hs_bf = nc.dram_tensor("hs_bf", [DP, M], BF16, kind="ExternalInput")      # hs.T
    wq_bf = nc.dram_tensor("wq_bf", [DP, RROWS], BF16, kind="ExternalInput")  # wq_loc.T
    wd_bf = nc.dram_tensor("wd_bf", [QPAD, D], BF16, kind="ExternalInput")    # wd rows for local heads
    cos_in = nc.dram_tensor("cos_in", [128, L], F32, kind="ExternalInput")
    sin_in = nc.dram_tensor("sin_in", [128, L], F32, kind="ExternalInput")
    tri_in = nc.dram_tensor("tri", [128, 128], BF16, kind="ExternalInput")
    prope_in = nc.dram_tensor("prope_in", [128, 128], F32R, kind="ExternalInput")
    id64_in = nc.dram_tensor("id64_in", [64, 64], F32R, kind="ExternalInput")
    ones_in = nc.dram_tensor("ones_in", [1, 64], F32R, kind="ExternalInput")
    colones_in = nc.dram_tensor("colones_in", [128, 16], BF16, kind="ExternalInput")
    out = nc.dram_tensor("out", [M, D], BF16, kind="ExternalOutput")
    if _DEBUG_DUMPS:
        dbg_qk = nc.dram_tensor("dbg_qk", [128, 5 * M], BF16, kind="ExternalOutput")
        dbg_attn = nc.dram_tensor("dbg_attn", [128, 5 * M], BF16, kind="ExternalOutput")
        dbg_vnat = nc.dram_tensor("dbg_vnat", [128, N * 8 * (DKV + 1)], BF16,
                                  kind="ExternalOutput")
        dbg_kd = nc.dram_tensor("dbg_kd", [128, N * L], BF16, kind="ExternalOutput")

    with tile.TileContext(nc) as tc, ExitStack() as top:
        constp = top.enter_context(tc.tile_pool(name="const", bufs=1))
        workp = top.enter_context(tc.tile_pool(name="work", bufs=2))
        otp = top.enter_context(tc.tile_pool(name="otp", bufs=4))
        expp = top.enter_context(tc.tile_pool(name="exps", bufs=8))
        psG = top.enter_context(tc.tile_pool(name="psG", bufs=2, space="PSUM"))
        psS = top.enter_context(tc.tile_pool(name="psS", bufs=2, space="PSUM"))
        psV = top.enter_context(tc.tile_pool(name="psV", bufs=3, space="PSUM"))
        psR = top.enter_context(tc.tile_pool(name="psR", bufs=1, space="PSUM"))

        # ---- constants ----
        cosb = constp.tile([128, L], F32)
        sinb = constp.tile([128, L], F32)
        trib = constp.tile([128, 128], BF16)
        prope = constp.tile([128, 128], F32R)
        id64 = constp.tile([64, 64], F32R)
        ones_1x64 = constp.tile([1, 64], F32R)
        nc.sync.dma_start(cosb[:], cos_in[:])
        nc.sync.dma_start(sinb[:], sin_in[:])
        nc.sync.dma_start(trib[:], tri_in[:])
        nc.sync.dma_start(prope[:], prope_in[:])
        nc.sync.dma_start(id64[:], id64_in[:])
        nc.sync.dma_start(ones_1x64[:], ones_in[:])

        # ---- persistent SBUF state ----
        bigp = top.enter_context(tc.tile_pool(name="big", bufs=1))
        qk_bf = bigp.tile([128, 5, M], BF16)          # rope'd q (+k in chunk 4 hi)
        attn_sb = bigp.tile([128, 5, M], BF16)        # attention out rows
        v_nat = bigp.tile([128, N * 8, DKV + 1], BF16)  # v.T per j-tile + ones col
        kT_dup = bigp.tile([128, N, L], BF16)         # k duplicated on both halves

        nc.sync.dma_start(v_nat[:, :, DKV:DKV + 1],
                          colones_in[:].rearrange("p (s o) -> p s o", o=1))
        nc.vector.memset(attn_sb[64:128, 4, :], 0.0)

        # stage A: QKV weights + hs chunks (region later reused for wd)
        stageA = ExitStack()
        wqp = stageA.enter_context(tc.tile_pool(name="wqp", bufs=1))
        hstp = stageA.enter_context(tc.tile_pool(name="hst", bufs=2))
        wqT = wqp.tile([128, KT, RROWS], BF16)
        wq_r = wq_bf[:].rearrange("(kt p) r -> p kt r", p=128)
        nc.sync.dma_start(wqT[:], wq_r[:])

        hs_r = hs_bf[:].rearrange("(kt p) m -> p kt m", p=128)

        # ---------- stage-1 unit generators (QKV + RoPE per chunk) ----------
        def qkv_chunk(mc):
            """Generator: QKV projection + RoPE + v prep for token chunk mc."""
            n, loc0 = mc // 4, MCH * (mc % 4)
            sl = slice(MCH * mc, MCH * (mc + 1))
            hsT = hstp.tile([128, KT, MCH], BF16, tag="hsT")
            nc.sync.dma_start(hsT[:], hs_r[:, :, sl])
            for rc in range(RC):
                rp = 128 if rc < 5 else 64
                ps = psG.tile([128, 512], F32, tag="g")
                for kt in range(KT):
                    nc.tensor.matmul(
                        ps[:rp, :MCH], wqT[:, kt, 128 * rc:128 * rc + rp],
                        hsT[:, kt, :], start=(kt == 0), stop=(kt == KT - 1))
                    if kt == KT // 2:
                        yield
                if rc < 5:
                    # rope: qk = x*cos + (P x)*sin
                    xb = workp.tile([128, MCH], F32R, tag="xb")
                    nc.scalar.activation(xb[:], ps[:, :MCH], COPYF)
                    pp = psS.tile([128, MCH], F32, tag="s")
                    nc.tensor.matmul(pp[:], prope[:], xb[:], start=True, stop=True)
                    a = workp.tile([128, MCH], F32, tag="ra")
                    nc.vector.tensor_mul(a[:], xb[:], cosb[:, loc0:loc0 + MCH])
                    b = workp.tile([128, MCH], F32, tag="rb")
                    nc.vector.tensor_mul(b[:], pp[:], sinb[:, loc0:loc0 + MCH])
                    nc.vector.tensor_add(qk_bf[:, rc, sl], a[:], b[:])
                else:
                    # v rows -> transposed into v_nat j-tiles
                    vsb = workp.tile([64, MCH], F32R, tag="vsb")
                    nc.scalar.activation(vsb[:], ps[0:64, :MCH], COPYF)
                    for half in range(2):
                        tp = psS.tile([128, 64], F32R, tag="s")
                        nc.tensor.transpose(
                            tp[:], vsb[:, 128 * half:128 * (half + 1)], id64[:])
                        nc.scalar.activation(
                            v_nat[:, 2 * mc + half, 0:DKV], tp[:], COPYF)
                yield
            if mc % 4 == 3:
                # k of batch n complete: build kT_dup (both halves)
                nsl = slice(L * n, L * (n + 1))
                nc.scalar.dma_start(kT_dup[0:64, n, :], qk_bf[64:128, 4, nsl])
                nc.scalar.dma_start(kT_dup[64:128, n, :], qk_bf[64:128, 4, nsl])
                yield

        # ---------- attention per head ----------
        def attn_head(n, h):
            """Generator: one attention head, yields between pipeline units."""
            poff = (64 * h) % 128
            prc = (64 * h) // 128
            kTn = kT_dup[poff:poff + 64, n, :]
            qh = qk_bf[poff:poff + 64, prc, L * n:L * (n + 1)]
            for qc in range(2):
                av = psV.tile([65, 512], F32, tag="av")
                njt = 4 * (qc + 1)
                pend = None
                for jt in range(njt):
                    off = max(0, 128 * jt - 512 * qc)
                    sp = psS.tile([128, 512], F32, tag="s")
                    nc.tensor.matmul(
                        sp[:, 0:512 - off],
                        kTn[:, 128 * jt:128 * (jt + 1)],
                        qh[:, 512 * qc + off:512 * (qc + 1)],
                        start=True, stop=True)
                    et = expp.tile([128, 512], BF16, tag="exp")
                    nc.scalar.activation(
                        et[:, off:512], sp[:, 0:512 - off], EXPF,
                        scale=1.0 / math.sqrt(DKV))
                    if 128 * jt >= 512 * qc:
                        nc.vector.tensor_mul(
                            et[:, off:off + 128], et[:, off:off + 128], trib[:])
                    if pend is not None:
                        pjt, po, pet = pend
                        nc.tensor.matmul(
                            av[:, po:512], v_nat[:, 8 * n + pjt, :], pet[:, po:512],
                            start=(pjt == 0), stop=False)
                    pend = (jt, off, et)
                    yield
                pjt, po, pet = pend
                nc.tensor.matmul(
                    av[:, po:512], v_nat[:, 8 * n + pjt, :], pet[:, po:512],
                    start=(pjt == 0), stop=True)
                den = workp.tile([1, 512], F32, tag="den")
                nc.vector.tensor_copy(den[:], av[64:65, :])
                rec32 = workp.tile([1, 512], F32, tag="rec32")
                with nc.allow_low_precision(reason="softmax denom"):
                    nc.vector.reciprocal_approx_fast(rec32[:], den[:])
                rec = workp.tile([1, 512], F32R, tag="rec")
                nc.vector.tensor_copy(rec[:], rec32[:])
                yield
                pr = psR.tile([64, 512], F32, tag="pp")
                nc.tensor.matmul(pr[:], ones_1x64[:], rec[:], start=True, stop=True)
                ob32 = workp.tile([64, 512], F32, tag="ob32")
                nc.scalar.activation(ob32[:], av[0:64, :], COPYF)
                yield
                nc.vector.tensor_mul(
                    attn_sb[poff:poff + 64, prc, L * n + 512 * qc:L * n + 512 * (qc + 1)],
                    ob32[:], pr[:])
                yield

        # ---------- dense units ----------
        CCH = [512] * 8 + [448]          # dense column chunks (sum = 4544)

        def dense_unit(wdT, n, mt, ci, col, w):
            pa = psG.tile([128, 512], F32, tag="g")
            for kt in range(QPAD // 128):
                nc.tensor.matmul(
                    pa[:, :w], attn_sb[:, kt, L * n + 128 * mt:L * n + 128 * (mt + 1)],
                    wdT[:, kt, col:col + w],
                    start=(kt == 0), stop=(kt == QPAD // 128 - 1))
            ot = otp.tile([128, 512], BF16, tag="ot")
            if ci % 2 == 0:
                nc.scalar.activation(ot[:, :w], pa[:, :w], COPYF)
            else:
                nc.vector.tensor_copy(ot[:, :w], pa[:, :w])
            nc.sync.dma_start(
                out[L * n + 128 * mt:L * n + 128 * (mt + 1), col:col + w],
                ot[:, :w])

        def dense_units(wdT, n):
            for mt in range(8):
                col = 0
                for ci, w in enumerate(CCH):
                    yield (lambda n=n, mt=mt, ci=ci, col=col, w=w:
                           dense_unit(wdT, n, mt, ci, col, w))
                    col += w

        # ---------- drivers ----------
        def gen_queue_fillers(gens, n_units):
            """n_units filler thunks advancing the shared generator queue."""
            q = list(gens)
            def unit():
                while q:
                    try:
                        next(q[0])
                        return
                    except StopIteration:
                        q.pop(0)
            return [unit] * n_units

        def run_heads(n, fillers):
            """Round-robin 3 in-flight heads; consume fillers uniformly."""
            nxt = 0
            slots = []
            for _ in range(min(3, HPC)):
                slots.append(attn_head(n, nxt))
                nxt += 1
            est_rounds = HPC * 18 // 3
            pace = len(fillers) / max(est_rounds, 1)
            acc = 0.0
            fi = 0
            while slots:
                for g in list(slots):
                    try:
                        next(g)
                    except StopIteration:
                        slots.remove(g)
                        if nxt < HPC:
                            slots.append(attn_head(n, nxt))
                            nxt += 1
                acc += pace
                while acc >= 1.0 and fi < len(fillers):
                    fillers[fi]()
                    fi += 1
                    acc -= 1.0
            while fi < len(fillers):
                fillers[fi]()
                fi += 1

        # stage 1: QKV+rope for batch 0 (chunks 0-3), sequential
        for mc in range(4):
            for _ in qkv_chunk(mc):
                pass

        # batch-0 attention with batch-1 QKV as PE filler
        run_heads(0, gen_queue_fillers(
            [qkv_chunk(mc) for mc in range(4, NMC)], 49))

        # wq/hs dead; reuse their SBUF region for wd (WAR deps auto-inserted)
        stageA.close()
        stageB = ExitStack()
        wdp = stageB.enter_context(tc.tile_pool(name="wdp", bufs=1))
        wdT = wdp.tile([128, QPAD // 128, D], BF16)
        wd_r = wd_bf[:].rearrange("(kt p) c -> p kt c", p=128)
        wcol = 0
        for w in CCH:
            nc.sync.dma_start(wdT[:, :, wcol:wcol + w], wd_r[:, :, wcol:wcol + w])
            wcol += w

        # batch-1 attention with batch-0 dense as PE filler
        run_heads(1, list(dense_units(wdT, 0)))

        # drain: batch-1 dense
        for u in dense_units(wdT, 1):
            u()
        stageB.close()

        if _DEBUG_DUMPS:
            nc.sync.dma_start(
                dbg_qk[:].rearrange("p (c m) -> p c m", c=5), qk_bf[:])
            nc.sync.dma_start(
                dbg_attn[:].rearrange("p (c m) -> p c m", c=5), attn_sb[:])
            nc.sync.dma_start(
                dbg_vnat[:].rearrange("p (j d) -> p j d", j=N * 8), v_nat[:])
            nc.sync.dma_start(
                dbg_kd[:].rearrange("p (n l) -> p n l", n=N), kT_dup[:])

    nc.compile()
    return nc


_NC_CACHE = None


def _get_nc():
    global _NC_CACHE
    if _NC_CACHE is None:
        _NC_CACHE = _build()
    return _NC_CACHE


def _host_inputs(hidden_states, w_qkv, w_dense):
    """Build the per-core input maps (transpose + slice + bf16 cast on host)."""
    hs = np.asarray(hidden_states, dtype=np.float32).reshape(M, D)
    w_qkv = np.asarray(w_qkv, dtype=np.float32)
    w_dense = np.asarray(w_dense, dtype=np.float32)
    hs_bf = np.zeros((DP, M), dtype=ml_dtypes.bfloat16)
    hs_bf[:D, :] = np.ascontiguousarray(hs.T).astype(ml_dtypes.bfloat16)

    # RoPE tables, transposed to [dkv, l], duplicated on partitions 0-63 / 64-127
    inv_freq = 1.0 / (ROPE_BASE ** (np.arange(0, DKV, 2, dtype=np.float32) / DKV))
    t = np.arange(L, dtype=np.float32)
    freqs = np.outer(t, inv_freq)
    emb = np.concatenate([freqs, freqs], axis=-1)        # [L, DKV]
    cosT = np.cos(emb).T.astype(np.float32)              # [DKV, L]
    sinT = np.sin(emb).T.astype(np.float32)
    cos2 = np.concatenate([cosT, cosT], axis=0)
    sin2 = np.concatenate([sinT, sinT], axis=0)

    # tri[j, q] = 1 if j <= q (within-tile causal mask)
    tri = (np.arange(128)[:, None] <= np.arange(128)[None, :]).astype(
        ml_dtypes.bfloat16)

    # RoPE rotation: (P x)[d] = -x[d+32] (d<32), x[d-32] (d>=32); lhsT = P.T, 2 blocks
    P1 = np.zeros((DKV, DKV), dtype=np.float32)
    for d in range(32):
        P1[d, d + 32] = -1.0
        P1[d + 32, d] = 1.0
    PT = P1.T
    prope2 = np.zeros((128, 128), dtype=np.float32)
    prope2[:64, :64] = PT
    prope2[64:, 64:] = PT

    ident64 = np.eye(64, dtype=np.float32)

    kv_bf = w_qkv[H * DKV:, :].T.astype(ml_dtypes.bfloat16)   # [D, 128]
    in_maps = []
    for c in range(NCORES):
        h0 = HPC * c
        nh = min(HPC, H - h0)
        wq_loc = np.zeros((DP, RROWS), dtype=ml_dtypes.bfloat16)
        wq_loc[:D, :nh * DKV] = w_qkv[h0 * DKV:(h0 + nh) * DKV, :].T.astype(
            ml_dtypes.bfloat16)
        wq_loc[:D, QROWS:] = kv_bf

        # dense weight rows for this core's heads: w_dense columns
        # [64*h0 : 64*(h0+nh)) transposed, zero-padded to QPAD rows
        wd_loc = np.zeros((QPAD, D), dtype=ml_dtypes.bfloat16)
        wd_loc[:nh * DKV, :] = w_dense[:, DKV * h0:DKV * (h0 + nh)].T.astype(
            ml_dtypes.bfloat16)

        in_maps.append({
            "hs_bf": hs_bf,
            "wq_bf": wq_loc,
            "wd_bf": wd_loc,
            "cos_in": cos2,
            "sin_in": sin2,
            "tri": tri,
            "prope_in": prope2,
            "id64_in": ident64,
            "ones_in": np.ones((1, 64), dtype=np.float32),
            "colones_in": np.ones((128, 16), dtype=ml_dtypes.bfloat16),
        })
    return in_maps


def kernel(hidden_states, w_qkv, w_dense, _trace=False, _trace_kwargs=None):
    nc = _get_nc()
    in_maps = _host_inputs(hidden_states, w_qkv, w_dense)
    kw = {}
    if _trace:
        kw = dict(trace=True, **(_trace_kwargs or {}))
    res = run_bass_kernel_spmd(nc, in_maps, list(range(NCORES)), **kw)
    full = res.results[0]["out"].astype(np.float32)
    for c in range(1, NCORES):
        full += res.results[c]["out"].astype(np.float32)
    kernel._last_exec_time_ns = res.exec_time_ns
    return full.reshape(N, L, D).astype(np.float32)


# revision 29
# speedup vs baseline: 1.5821x; 1.0144x over previous
"""Falcon-style MQA attention (71 heads, 1 KV head, RoPE, causal) on 8 TRN2 NeuronCores.

Sharding: tensor-parallel over query heads (9 slots per core, core 7 has 8 real
+ 1 zero-pad), single KV head replicated. Per core: QKV projection for its
heads (+KV), RoPE, causal attention in transposed [k, q] layout, then a PARTIAL
dense projection over the core's own head rows for all 4544 output columns.
The host sums the 8 bf16 partial outputs (contraction-sharded dense).

v2: software-pipelined to keep the tensor engine continuously busy (HAM warm):
 - QKV+RoPE pipelined per 256-token chunk
 - remaining QKV / dense GEMMs interleaved as PE filler inside attention
 - reciprocal_approx_fast for softmax denominators (was 3.3us/call reciprocal)
 - bf16 moving operands on all hot matmuls; elementwise spread over Act/DVE/GpSimd
 - coarse 3D DMAs; bf16 output partials summed on host
 - wq/hs SBUF region reused for wd (staged pools)

Self-contained: hardcodes all shapes; needs only numpy + ml_dtypes + concourse.
"""

import math
from contextlib import ExitStack

import numpy as np
import ml_dtypes

import concourse.bass as bass
import concourse.mybir as mybir
import concourse.tile as tile
from concourse import bacc
from concourse.bass_utils import run_bass_kernel_spmd

NCORES = 8
N, L, D = 2, 1024, 4544
H, DKV = 71, 64
M = N * L                    # 2048 tokens
DP = 4608                    # D padded to 36*128
KT = DP // 128               # 36 contraction tiles for QKV
HPC = 9                      # head slots per core (core 7: 8 real + 1 zero-pad)
QROWS = HPC * DKV            # 576 q rows per core
QPAD = 640                   # padded to 5*128 for the dense contraction
RROWS = QROWS + 2 * DKV      # 704 fused rows per core (q + k + v)
RC = 6                       # row-chunks of the QKV output (5x128 + 64)
MCH = 512                    # QKV token-chunk width (4 chunks)
NMC = M // MCH
CPB = NMC // N               # chunks per batch
JPC = MCH // 128             # j-tiles per chunk
ROPE_BASE = 10000.0

F32 = mybir.dt.float32
F32R = mybir.dt.float32r
BF16 = mybir.dt.bfloat16
COPYF = mybir.ActivationFunctionType.Copy
EXPF = mybir.ActivationFunctionType.Exp

_DEBUG_DUMPS = False


def _build():
    nc = bacc.Bacc("TRN2", target_bir_lowering=False, debug=False, num_devices=NCORES)

    hs_bf = nc.dram_tensor("hs_bf", [DP, M], BF16, kind="ExternalInput")      # hs.T
    wq_bf = nc.dram_tensor("wq_bf", [DP, RROWS], BF16, kind="ExternalInput")  # wq_loc.T
    wd_bf = nc.dram_tensor("wd_bf", [QPAD, D], BF16, kind="ExternalInput")    # wd rows for local heads
    cos_in = nc.dram_tensor("cos_in", [128, L], F32, kind="ExternalInput")
    sin_in = nc.dram_tensor("sin_in", [128, L], F32, kind="ExternalInput")
    tri_in = nc.dram_tensor("tri", [128, 128], BF16, kind="ExternalInput")
    prope_in = nc.dram_tensor("prope_in", [128, 128], F32R, kind="ExternalInput")
    id64_in = nc.dram_tensor("id64_in", [64, 64], F32R, kind="ExternalInput")
    ones_in = nc.dram_tensor("ones_in", [1, 64], F32R, kind="ExternalInput")
    colones_in = nc.dram_tensor("colones_in", [128, 16], BF16, kind="ExternalInput")
    out = nc.dram_tensor("out", [M, D], BF16, kind="ExternalOutput")
    if _DEBUG_DUMPS:
        dbg_qk = nc.dram_tensor("dbg_qk", [128, 5 * M], BF16, kind="ExternalOutput")
        dbg_attn = nc.dram_tensor("dbg_attn", [128, 5 * M], BF16, kind="ExternalOutput")
        dbg_vnat = nc.dram_tensor("dbg_vnat", [128, N * 8 * (DKV + 1)], BF16,
                                  kind="ExternalOutput")
        dbg_kd = nc.dram_tensor("dbg_kd", [128, N * L], BF16, kind="ExternalOutput")

    with tile.TileContext(nc) as tc, ExitStack() as top:
        constp = top.enter_context(tc.tile_pool(name="const", bufs=1))
        workp = top.enter_context(tc.tile_pool(name="work", bufs=2))
        finp = top.enter_context(tc.tile_pool(name="fin", bufs=1))
        otp = top.enter_context(tc.tile_pool(name="otp", bufs=3))
        expp = top.enter_context(tc.tile_pool(name="exps", bufs=6))
        psG = top.enter_context(tc.tile_pool(name="psG", bufs=2, space="PSUM"))
        psS = top.enter_context(tc.tile_pool(name="psS", bufs=2, space="PSUM"))
        psV = top.enter_context(tc.tile_pool(name="psV", bufs=3, space="PSUM"))
        psR = top.enter_context(tc.tile_pool(name="psR", bufs=1, space="PSUM"))

        # ---- constants ----
        cosb = constp.tile([128, L], F32)
        sinb = constp.tile([128, L], F32)
        trib = constp.tile([128, 128], BF16)
        prope = constp.tile([128, 128], F32R)
        id64 = constp.tile([64, 64], F32R)
        ones_1x64 = constp.tile([1, 64], F32R)
        nc.sync.dma_start(cosb[:], cos_in[:])
        nc.sync.dma_start(sinb[:], sin_in[:])
        nc.sync.dma_start(trib[:], tri_in[:])
        nc.sync.dma_start(prope[:], prope_in[:])
        nc.sync.dma_start(id64[:], id64_in[:])
        nc.sync.dma_start(ones_1x64[:], ones_in[:])

        # ---- persistent SBUF state ----
        bigp = top.enter_context(tc.tile_pool(name="big", bufs=1))
        qk_bf = bigp.tile([128, 5, M], BF16)          # rope'd q (+k in chunk 4 hi)
        attn_sb = bigp.tile([128, 5, M], BF16)        # attention out rows
        v_nat = bigp.tile([128, N * 8, DKV + 1], BF16)  # v.T per j-tile + ones col
        kT_dup = bigp.tile([128, N, L], BF16)         # k duplicated on both halves

        nc.sync.dma_start(v_nat[:, :, DKV:DKV + 1],
                          colones_in[:].rearrange("p (s o) -> p s o", o=1))
        nc.vector.memset(attn_sb[64:128, 4, :], 0.0)

        # stage A: QKV weights + hs chunks (region later reused for wd)
        stageA = ExitStack()
        wqp = stageA.enter_context(tc.tile_pool(name="wqp", bufs=1))
        hstp = stageA.enter_context(tc.tile_pool(name="hst", bufs=2))
        wqT = wqp.tile([128, KT, RROWS], BF16)
        wq_r = wq_bf[:].rearrange("(kt p) r -> p kt r", p=128)
        for k0 in range(0, KT, 9):
            nc.sync.dma_start(wqT[:, k0:k0 + 9, :], wq_r[:, k0:k0 + 9, :])

        hs_r = hs_bf[:].rearrange("(kt p) m -> p kt m", p=128)

        # ---------- stage-1 unit generators (QKV + RoPE per chunk) ----------
        def qkv_chunk(mc):
            """Generator: QKV projection + RoPE + v prep for token chunk mc."""
            n, loc0 = mc // CPB, MCH * (mc % CPB)
            sl = slice(MCH * mc, MCH * (mc + 1))
            hsT = hstp.tile([128, KT, MCH], BF16, tag="hsT")
            for k0 in range(0, KT, 9):
                nc.sync.dma_start(hsT[:, k0:k0 + 9, :], hs_r[:, k0:k0 + 9, sl])
            for rc in range(RC):
                rp = 128 if rc < 5 else 64
                ps = psG.tile([128, 512], F32, tag="g")
                for kt in range(KT):
                    nc.tensor.matmul(
                        ps[:rp, :MCH], wqT[:, kt, 128 * rc:128 * rc + rp],
                        hsT[:, kt, :], start=(kt == 0), stop=(kt == KT - 1))
                    if kt % 12 == 11 and kt != KT - 1:
                        yield
                if rc < 5:
                    # rope: qk = x*cos + (P x)*sin  (256-wide halves)
                    for hh in range(MCH // 256):
                        hsl = slice(256 * hh, 256 * (hh + 1))
                        gsl = slice(MCH * mc + 256 * hh, MCH * mc + 256 * (hh + 1))
                        csl = slice(loc0 + 256 * hh, loc0 + 256 * (hh + 1))
                        xb = workp.tile([128, 256], F32R, tag="xb")
                        nc.scalar.activation(xb[:], ps[:, hsl], COPYF)
                        pp = psS.tile([128, 256], F32, tag="s")
                        nc.tensor.matmul(pp[:], prope[:], xb[:],
                                         start=True, stop=True)
                        a = workp.tile([128, 256], F32, tag="ra")
                        nc.vector.tensor_mul(a[:], xb[:], cosb[:, csl])
                        b = workp.tile([128, 256], F32, tag="rb")
                        nc.vector.tensor_mul(b[:], pp[:], sinb[:, csl])
                        nc.vector.tensor_add(qk_bf[:, rc, gsl], a[:], b[:])
                else:
                    # v rows -> transposed into v_nat j-tiles
                    for hh in range(MCH // 256):
                        vsb = workp.tile([64, 256], F32R, tag="vsb")
                        nc.scalar.activation(
                            vsb[:], ps[0:64, 256 * hh:256 * (hh + 1)], COPYF)
                        for half in range(2):
                            tp = psS.tile([128, 64], F32R, tag="s")
                            nc.tensor.transpose(
                                tp[:], vsb[:, 128 * half:128 * (half + 1)], id64[:])
                            nc.scalar.activation(
                                v_nat[:, JPC * mc + 2 * hh + half, 0:DKV],
                                tp[:], COPYF)
                yield
            if mc % CPB == CPB - 1:
                # k of batch n complete: build kT_dup (both halves)
                nsl = slice(L * n, L * (n + 1))
                nc.scalar.dma_start(kT_dup[0:64, n, :], qk_bf[64:128, 4, nsl])
                nc.scalar.dma_start(kT_dup[64:128, n, :], qk_bf[64:128, 4, nsl])
                yield

        # ---------- attention per head ----------
        def attn_head(n, h):
            """Generator: one attention head, yields between pipeline units."""
            poff = (64 * h) % 128
            prc = (64 * h) // 128
            kTn = kT_dup[poff:poff + 64, n, :]
            qh = qk_bf[poff:poff + 64, prc, L * n:L * (n + 1)]
            for qc in range(2):
                av = psV.tile([65, 512], F32, tag="av")
                njt = 4 * (qc + 1)
                pend = None
                for jt in range(njt):
                    off = max(0, 128 * jt - 512 * qc)
                    sp = psS.tile([128, 512], F32, tag="s")
                    nc.tensor.matmul(
                        sp[:, 0:512 - off],
                        kTn[:, 128 * jt:128 * (jt + 1)],
                        qh[:, 512 * qc + off:512 * (qc + 1)],
                        start=True, stop=True)
                    et = expp.tile([128, 512], BF16, tag="exp")
                    nc.scalar.activation(
                        et[:, off:512], sp[:, 0:512 - off], EXPF,
                        scale=1.0 / math.sqrt(DKV))
                    if 128 * jt >= 512 * qc:
                        nc.vector.tensor_mul(
                            et[:, off:off + 128], et[:, off:off + 128], trib[:])
                    if pend is not None:
                        pjt, po, pet = pend
                        nc.tensor.matmul(
                            av[:, po:512], v_nat[:, 8 * n + pjt, :], pet[:, po:512],
                            start=(pjt == 0), stop=False)
                    pend = (jt, off, et)
                    yield
                pjt, po, pet = pend
                nc.tensor.matmul(
                    av[:, po:512], v_nat[:, 8 * n + pjt, :], pet[:, po:512],
                    start=(pjt == 0), stop=True)
                den = finp.tile([1, 512], F32, tag="den")
                nc.vector.tensor_copy(den[:], av[64:65, :])
                rec32 = finp.tile([1, 512], F32, tag="rec32")
                with nc.allow_low_precision(reason="softmax denom"):
                    nc.vector.reciprocal_approx_fast(rec32[:], den[:])
                rec = finp.tile([1, 512], F32R, tag="rec")
                nc.vector.tensor_copy(rec[:], rec32[:])
                yield
                pr = psR.tile([64, 512], F32, tag="pp")
                nc.tensor.matmul(pr[:], ones_1x64[:], rec[:], start=True, stop=True)
                ob32 = finp.tile([64, 512], F32, tag="ob32")
                nc.scalar.activation(ob32[:], av[0:64, :], COPYF)
                yield
                nc.vector.tensor_mul(
                    attn_sb[poff:poff + 64, prc, L * n + 512 * qc:L * n + 512 * (qc + 1)],
                    ob32[:], pr[:])
                yield

        # ---------- dense units ----------
        CCH = [512] * 8 + [448]          # dense column chunks (sum = 4544)

        def dense_unit(wdT, n, mt, ci, col, w):
            pa = psG.tile([128, 512], F32, tag="g")
            for kt in range(QPAD // 128):
                nc.tensor.matmul(
                    pa[:, :w], attn_sb[:, kt, L * n + 128 * mt:L * n + 128 * (mt + 1)],
                    wdT[:, kt, col:col + w],
                    start=(kt == 0), stop=(kt == QPAD // 128 - 1))
            ot = otp.tile([128, 512], BF16, tag="ot")
            if ci % 2 == 0:
                nc.scalar.activation(ot[:, :w], pa[:, :w], COPYF)
            else:
                nc.vector.tensor_copy(ot[:, :w], pa[:, :w])
            nc.sync.dma_start(
                out[L * n + 128 * mt:L * n + 128 * (mt + 1), col:col + w],
                ot[:, :w])

        def dense_units(wdT, n):
            for mt in range(8):
                col = 0
                for ci, w in enumerate(CCH):
                    yield (lambda n=n, mt=mt, ci=ci, col=col, w=w:
                           dense_unit(wdT, n, mt, ci, col, w))
                    col += w

        # ---------- drivers ----------
        def gen_queue_fillers(gens, n_units):
            """n_units filler thunks advancing the shared generator queue."""
            q = list(gens)
            def unit():
                while q:
                    try:
                        next(q[0])
                        return
                    except StopIteration:
                        q.pop(0)
            return [unit] * n_units

        def run_heads(n, fillers):
            """Round-robin 3 in-flight heads; consume fillers uniformly."""
            nxt = 0
            slots = []
            for _ in range(min(3, HPC)):
                slots.append(attn_head(n, nxt))
                nxt += 1
            est_rounds = HPC * 18 // 3
            pace = len(fillers) / max(est_rounds, 1)
            acc = 0.0
            fi = 0
            while slots:
                for g in list(slots):
                    try:
                        next(g)
                    except StopIteration:
                        slots.remove(g)
                        if nxt < HPC:
                            slots.append(attn_head(n, nxt))
                            nxt += 1
                acc += pace
                while acc >= 1.0 and fi < len(fillers):
                    fillers[fi]()
                    fi += 1
                    acc -= 1.0
            while fi < len(fillers):
                fillers[fi]()
                fi += 1

        # stage 1: QKV+rope for batch 0, sequential
        for mc in range(CPB):
            for _ in qkv_chunk(mc):
                pass

        # batch-0 attention with batch-1 QKV as PE filler
        run_heads(0, gen_queue_fillers(
            [qkv_chunk(mc) for mc in range(CPB, NMC)], 37))

        # wq/hs dead; reuse their SBUF region for wd (WAR deps auto-inserted)
        stageA.close()
        stageB = ExitStack()
        wdp = stageB.enter_context(tc.tile_pool(name="wdp", bufs=1))
        wdT = wdp.tile([128, QPAD // 128, D], BF16)
        wd_r = wd_bf[:].rearrange("(kt p) c -> p kt c", p=128)
        wcol = 0
        for w in CCH:
            nc.sync.dma_start(wdT[:, :, wcol:wcol + w], wd_r[:, :, wcol:wcol + w])
            wcol += w

        # batch-1 attention with batch-0 dense as PE filler
        run_heads(1, list(dense_units(wdT, 0)))

        # drain: batch-1 dense
        for u in dense_units(wdT, 1):
            u()
        stageB.close()

        if _DEBUG_DUMPS:
            nc.sync.dma_start(
                dbg_qk[:].rearrange("p (c m) -> p c m", c=5), qk_bf[:])
            nc.sync.dma_start(
                dbg_attn[:].rearrange("p (c m) -> p c m", c=5), attn_sb[:])
            nc.sync.dma_start(
                dbg_vnat[:].rearrange("p (j d) -> p j d", j=N * 8), v_nat[:])
            nc.sync.dma_start(
                dbg_kd[:].rearrange("p (n l) -> p n l", n=N), kT_dup[:])

    nc.compile()
    return nc


_NC_CACHE = None


def _get_nc():
    global _NC_CACHE
    if _NC_CACHE is None:
        _NC_CACHE = _build()
    return _NC_CACHE


def _host_inputs(hidden_states, w_qkv, w_dense):
    """Build the per-core input maps (transpose + slice + bf16 cast on host)."""
    hs = np.asarray(hidden_states, dtype=np.float32).reshape(M, D)
    w_qkv = np.asarray(w_qkv, dtype=np.float32)
    w_dense = np.asarray(w_dense, dtype=np.float32)
    hs_bf = np.zeros((DP, M), dtype=ml_dtypes.bfloat16)
    hs_bf[:D, :] = np.ascontiguousarray(hs.T).astype(ml_dtypes.bfloat16)

    # RoPE tables, transposed to [dkv, l], duplicated on partitions 0-63 / 64-127
    inv_freq = 1.0 / (ROPE_BASE ** (np.arange(0, DKV, 2, dtype=np.float32) / DKV))
    t = np.arange(L, dtype=np.float32)
    freqs = np.outer(t, inv_freq)
    emb = np.concatenate([freqs, freqs], axis=-1)        # [L, DKV]
    cosT = np.cos(emb).T.astype(np.float32)              # [DKV, L]
    sinT = np.sin(emb).T.astype(np.float32)
    cos2 = np.concatenate([cosT, cosT], axis=0)
    sin2 = np.concatenate([sinT, sinT], axis=0)

    # tri[j, q] = 1 if j <= q (within-tile causal mask)
    tri = (np.arange(128)[:, None] <= np.arange(128)[None, :]).astype(
        ml_dtypes.bfloat16)

    # RoPE rotation: (P x)[d] = -x[d+32] (d<32), x[d-32] (d>=32); lhsT = P.T, 2 blocks
    P1 = np.zeros((DKV, DKV), dtype=np.float32)
    for d in range(32):
        P1[d, d + 32] = -1.0
        P1[d + 32, d] = 1.0
    PT = P1.T
    prope2 = np.zeros((128, 128), dtype=np.float32)
    prope2[:64, :64] = PT
    prope2[64:, 64:] = PT

    ident64 = np.eye(64, dtype=np.float32)

    kv_bf = w_qkv[H * DKV:, :].T.astype(ml_dtypes.bfloat16)   # [D, 128]
    in_maps = []
    for c in range(NCORES):
        h0 = HPC * c
        nh = min(HPC, H - h0)
        wq_loc = np.zeros((DP, RROWS), dtype=ml_dtypes.bfloat16)
        wq_loc[:D, :nh * DKV] = w_qkv[h0 * DKV:(h0 + nh) * DKV, :].T.astype(
            ml_dtypes.bfloat16)
        wq_loc[:D, QROWS:] = kv_bf

        # dense weight rows for this core's heads: w_dense columns
        # [64*h0 : 64*(h0+nh)) transposed, zero-padded to QPAD rows
        wd_loc = np.zeros((QPAD, D), dtype=ml_dtypes.bfloat16)
        wd_loc[:nh * DKV, :] = w_dense[:, DKV * h0:DKV * (h0 + nh)].T.astype(
            ml_dtypes.bfloat16)

        in_maps.append({
            "hs_bf": hs_bf,
            "wq_bf": wq_loc,
            "wd_bf": wd_loc,
            "cos_in": cos2,
            "sin_in": sin2,
            "tri": tri,
            "prope_in": prope2,
            "id64_in": ident64,
            "ones_in": np.ones((1, 64), dtype=np.float32),
            "colones_in": np.ones((128, 16), dtype=ml_dtypes.bfloat16),
        })
    return in_maps


def kernel(hidden_states, w_qkv, w_dense, _trace=False, _trace_kwargs=None):
    nc = _get_nc()
    in_maps = _host_inputs(hidden_states, w_qkv, w_dense)
    kw = {}
    if _trace:
        kw = dict(trace=True, **(_trace_kwargs or {}))
    res = run_bass_kernel_spmd(nc, in_maps, list(range(NCORES)), **kw)
    full = res.results[0]["out"].astype(np.float32)
    for c in range(1, NCORES):
        full += res.results[c]["out"].astype(np.float32)
    kernel._last_exec_time_ns = res.exec_time_ns
    return full.reshape(N, L, D).astype(np.float32)


# revision 36
# speedup vs baseline: 1.6228x; 1.0258x over previous
"""Falcon-style MQA attention (71 heads, 1 KV head, RoPE, causal) on 8 TRN2 NeuronCores.

Sharding: tensor-parallel over query heads (9 slots per core, core 7 has 8 real
+ 1 zero-pad), single KV head replicated. Per core: QKV projection for its
heads (+KV), RoPE, causal attention in transposed [k, q] layout, then a PARTIAL
dense projection over the core's own head rows for all 4544 output columns.
The host sums the 8 bf16 partial outputs (contraction-sharded dense).

v2: software-pipelined to keep the tensor engine continuously busy (HAM warm):
 - QKV+RoPE pipelined per 256-token chunk
 - remaining QKV / dense GEMMs interleaved as PE filler inside attention
 - reciprocal_approx_fast for softmax denominators (was 3.3us/call reciprocal)
 - bf16 moving operands on all hot matmuls; elementwise spread over Act/DVE/GpSimd
 - coarse 3D DMAs; bf16 output partials summed on host
 - wq/hs SBUF region reused for wd (staged pools)

Self-contained: hardcodes all shapes; needs only numpy + ml_dtypes + concourse.
"""

import math
from contextlib import ExitStack

import numpy as np
import ml_dtypes

import concourse.bass as bass
import concourse.mybir as mybir
import concourse.tile as tile
from concourse import bacc
from concourse.bass_utils import run_bass_kernel_spmd

NCORES = 8
N, L, D = 2, 1024, 4544
H, DKV = 71, 64
M = N * L                    # 2048 tokens
DP = 4608                    # D padded to 36*128
KT = DP // 128               # 36 contraction tiles for QKV
HPC = 9                      # head slots per core (core 7: 8 real + 1 zero-pad)
QROWS = HPC * DKV            # 576 q rows per core
QPAD = 640                   # padded to 5*128 for the dense contraction
RROWS = QROWS + 2 * DKV      # 704 fused rows per core (q + k + v)
RC = 6                       # row-chunks of the QKV output (5x128 + 64)
MCH = 512                    # QKV token-chunk width (4 chunks)
NMC = M // MCH
CPB = NMC // N               # chunks per batch
JPC = MCH // 128             # j-tiles per chunk
ROPE_BASE = 10000.0

F32 = mybir.dt.float32
F32R = mybir.dt.float32r
BF16 = mybir.dt.bfloat16
COPYF = mybir.ActivationFunctionType.Copy
EXPF = mybir.ActivationFunctionType.Exp

_DEBUG_DUMPS = False


def _build():
    nc = bacc.Bacc("TRN2", target_bir_lowering=False, debug=False, num_devices=NCORES)

    hs_bf = nc.dram_tensor("hs_bf", [DP, M], BF16, kind="ExternalInput")      # hs.T
    wq_bf = nc.dram_tensor("wq_bf", [DP, RROWS], BF16, kind="ExternalInput")  # wq_loc.T
    wd_bf = nc.dram_tensor("wd_bf", [QPAD, D], BF16, kind="ExternalInput")    # wd rows for local heads
    cos_in = nc.dram_tensor("cos_in", [128, L], F32, kind="ExternalInput")
    sin_in = nc.dram_tensor("sin_in", [128, L], F32, kind="ExternalInput")
    tri_in = nc.dram_tensor("tri", [128, 128], BF16, kind="ExternalInput")
    prope_in = nc.dram_tensor("prope_in", [128, 128], F32R, kind="ExternalInput")
    id64_in = nc.dram_tensor("id64_in", [64, 64], F32R, kind="ExternalInput")
    ones_in = nc.dram_tensor("ones_in", [1, 64], F32R, kind="ExternalInput")
    colones_in = nc.dram_tensor("colones_in", [128, 16], BF16, kind="ExternalInput")
    out = nc.dram_tensor("out", [M, D], BF16, kind="ExternalOutput")
    if _DEBUG_DUMPS:
        dbg_qk = nc.dram_tensor("dbg_qk", [128, 5 * M], BF16, kind="ExternalOutput")
        dbg_attn = nc.dram_tensor("dbg_attn", [128, 5 * M], BF16, kind="ExternalOutput")
        dbg_vnat = nc.dram_tensor("dbg_vnat", [128, N * 8 * (DKV + 1)], BF16,
                                  kind="ExternalOutput")
        dbg_kd = nc.dram_tensor("dbg_kd", [128, N * L], BF16, kind="ExternalOutput")

    with tile.TileContext(nc) as tc, ExitStack() as top:
        constp = top.enter_context(tc.tile_pool(name="const", bufs=1))
        workp = top.enter_context(tc.tile_pool(name="work", bufs=2))
        finp = top.enter_context(tc.tile_pool(name="fin", bufs=1))
        otp = top.enter_context(tc.tile_pool(name="otp", bufs=3))
        expp = top.enter_context(tc.tile_pool(name="exps", bufs=6))
        psG = top.enter_context(tc.tile_pool(name="psG", bufs=2, space="PSUM"))
        psS = top.enter_context(tc.tile_pool(name="psS", bufs=3, space="PSUM"))
        psV = top.enter_context(tc.tile_pool(name="psV", bufs=2, space="PSUM"))
        psR = top.enter_context(tc.tile_pool(name="psR", bufs=1, space="PSUM"))

        # ---- constants ----
        cosb = constp.tile([128, L], F32)
        sinb = constp.tile([128, L], F32)
        trib = constp.tile([128, 128], BF16)
        prope = constp.tile([128, 128], F32R)
        id64 = constp.tile([64, 64], F32R)
        ones_1x64 = constp.tile([1, 64], F32R)
        nc.sync.dma_start(cosb[:], cos_in[:])
        nc.sync.dma_start(sinb[:], sin_in[:])
        nc.sync.dma_start(trib[:], tri_in[:])
        nc.sync.dma_start(prope[:], prope_in[:])
        nc.sync.dma_start(id64[:], id64_in[:])
        nc.sync.dma_start(ones_1x64[:], ones_in[:])

        # ---- persistent SBUF state ----
        bigp = top.enter_context(tc.tile_pool(name="big", bufs=1))
        qk_bf = bigp.tile([128, 5, M], BF16)          # rope'd q (+k in chunk 4 hi)
        attn_sb = bigp.tile([128, 5, M], BF16)        # attention out rows
        v_nat = bigp.tile([128, N * 8, DKV + 1], BF16)  # v.T per j-tile + ones col
        kT_dup = bigp.tile([128, N, L], BF16)         # k duplicated on both halves

        nc.sync.dma_start(v_nat[:, :, DKV:DKV + 1],
                          colones_in[:].rearrange("p (s o) -> p s o", o=1))
        nc.vector.memset(attn_sb[64:128, 4, :], 0.0)

        # stage A: QKV weights + hs chunks (region later reused for wd)
        stageA = ExitStack()
        wqp = stageA.enter_context(tc.tile_pool(name="wqp", bufs=1))
        hstp = stageA.enter_context(tc.tile_pool(name="hst", bufs=2))
        wqT = wqp.tile([128, KT, RROWS], BF16)
        wq_r = wq_bf[:].rearrange("(kt p) r -> p kt r", p=128)
        for k0 in range(0, KT, 9):
            nc.sync.dma_start(wqT[:, k0:k0 + 9, :], wq_r[:, k0:k0 + 9, :])

        hs_r = hs_bf[:].rearrange("(kt p) m -> p kt m", p=128)

        # ---------- stage-1 unit generators (QKV + RoPE per chunk) ----------
        def qkv_chunk(mc):
            """Generator: QKV projection + RoPE + v prep for token chunk mc."""
            n, loc0 = mc // CPB, MCH * (mc % CPB)
            sl = slice(MCH * mc, MCH * (mc + 1))
            hsT = hstp.tile([128, KT, MCH], BF16, tag="hsT")
            for k0 in range(0, KT, 9):
                nc.sync.dma_start(hsT[:, k0:k0 + 9, :], hs_r[:, k0:k0 + 9, sl])
            for rc in range(RC):
                rp = 128 if rc < 5 else 64
                ps = psG.tile([128, 512], F32, tag="g")
                for kt in range(KT):
                    nc.tensor.matmul(
                        ps[:rp, :MCH], wqT[:, kt, 128 * rc:128 * rc + rp],
                        hsT[:, kt, :], start=(kt == 0), stop=(kt == KT - 1))
                    if kt % 12 == 11 and kt != KT - 1:
                        yield
                if rc < 5:
                    # rope: qk = x*cos + (P x)*sin  (256-wide halves)
                    for hh in range(MCH // 256):
                        hsl = slice(256 * hh, 256 * (hh + 1))
                        gsl = slice(MCH * mc + 256 * hh, MCH * mc + 256 * (hh + 1))
                        csl = slice(loc0 + 256 * hh, loc0 + 256 * (hh + 1))
                        xb = workp.tile([128, 256], F32R, tag="xb")
                        nc.scalar.activation(xb[:], ps[:, hsl], COPYF)
                        pp = psS.tile([128, 256], F32, tag="s")
                        nc.tensor.matmul(pp[:], prope[:], xb[:],
                                         start=True, stop=True)
                        a = workp.tile([128, 256], F32, tag="ra")
                        nc.vector.tensor_mul(a[:], xb[:], cosb[:, csl])
                        b = workp.tile([128, 256], F32, tag="rb")
                        nc.vector.tensor_mul(b[:], pp[:], sinb[:, csl])
                        nc.vector.tensor_add(qk_bf[:, rc, gsl], a[:], b[:])
                else:
                    # v rows -> transposed into v_nat j-tiles
                    for hh in range(MCH // 256):
                        vsb = workp.tile([64, 256], F32R, tag="vsb")
                        nc.scalar.activation(
                            vsb[:], ps[0:64, 256 * hh:256 * (hh + 1)], COPYF)
                        for half in range(2):
                            tp = psS.tile([128, 64], F32R, tag="s")
                            nc.tensor.transpose(
                                tp[:], vsb[:, 128 * half:128 * (half + 1)], id64[:])
                            nc.scalar.activation(
                                v_nat[:, JPC * mc + 2 * hh + half, 0:DKV],
                                tp[:], COPYF)
                yield
            if mc % CPB == CPB - 1:
                # k of batch n complete: build kT_dup (both halves)
                nsl = slice(L * n, L * (n + 1))
                nc.scalar.dma_start(kT_dup[0:64, n, :], qk_bf[64:128, 4, nsl])
                nc.scalar.dma_start(kT_dup[64:128, n, :], qk_bf[64:128, 4, nsl])
                yield

        # ---------- attention per head ----------
        def attn_head(n, h):
            """Generator: one attention head, fine-grained yields. Score (A)
            and AV (B) emissions yield separately so a lockstep pair driver
            lands both heads' 64-contraction score matmuls adjacent in the PE
            queue (row-groups 0-1 / 2-3 run concurrently)."""
            poff = (64 * h) % 128
            prc = (64 * h) // 128
            kTn = kT_dup[poff:poff + 64, n, :]
            qh = qk_bf[poff:poff + 64, prc, L * n:L * (n + 1)]
            for qc in range(2):
                av = psV.tile([65, 512], F32, tag="av")
                njt = 4 * (qc + 1)
                pend = None
                for jt in range(njt):
                    off = max(0, 128 * jt - 512 * qc)
                    sp = psS.tile([128, 512], F32, tag="s")
                    nc.tensor.matmul(
                        sp[:, 0:512 - off],
                        kTn[:, 128 * jt:128 * (jt + 1)],
                        qh[:, 512 * qc + off:512 * (qc + 1)],
                        start=True, stop=True)
                    et = expp.tile([128, 512], BF16, tag="exp")
                    nc.scalar.activation(
                        et[:, off:512], sp[:, 0:512 - off], EXPF,
                        scale=1.0 / math.sqrt(DKV))
                    if 128 * jt >= 512 * qc:
                        nc.vector.tensor_mul(
                            et[:, off:off + 128], et[:, off:off + 128], trib[:])
                    yield
                    if pend is not None:
                        pjt, po, pet = pend
                        nc.tensor.matmul(
                            av[:, po:512], v_nat[:, 8 * n + pjt, :], pet[:, po:512],
                            start=(pjt == 0), stop=False)
                    pend = (jt, off, et)
                    yield
                pjt, po, pet = pend
                nc.tensor.matmul(
                    av[:, po:512], v_nat[:, 8 * n + pjt, :], pet[:, po:512],
                    start=(pjt == 0), stop=True)
                den = finp.tile([1, 512], F32, tag="den")
                nc.vector.tensor_copy(den[:], av[64:65, :])
                rec32 = finp.tile([1, 512], F32, tag="rec32")
                with nc.allow_low_precision(reason="softmax denom"):
                    nc.vector.reciprocal_approx_fast(rec32[:], den[:])
                rec = finp.tile([1, 512], F32R, tag="rec")
                nc.vector.tensor_copy(rec[:], rec32[:])
                yield
                pr = psR.tile([64, 512], F32, tag="pp")
                nc.tensor.matmul(pr[:], ones_1x64[:], rec[:], start=True, stop=True)
                ob32 = finp.tile([64, 512], F32, tag="ob32")
                nc.scalar.activation(ob32[:], av[0:64, :], COPYF)
                osl = slice(L * n + 512 * qc, L * n + 512 * (qc + 1))
                nc.vector.tensor_mul(
                    attn_sb[poff:poff + 64, prc, osl], ob32[:], pr[:])
                if h == 8:
                    # replicate head-8 rows into the pad partitions for the
                    # paired dense chunk-4 matmuls
                    nc.vector.tensor_copy(
                        attn_sb[64:128, 4, osl], attn_sb[0:64, 4, osl])
                yield

        def attn_pair(n, h0, h1):
            """Lockstep pair driver: both heads advance one unit per round."""
            g0, g1 = attn_head(n, h0), attn_head(n, h1)
            while True:
                r0 = next(g0, "end")
                r1 = next(g1, "end")
                if r0 == "end" and r1 == "end":
                    return
                yield

        # ---------- dense units ----------
        CCH = [512] * 8 + [448]          # dense column chunks (sum = 4544)

        def dense_unit(wdT, n, mp, col, w):
            """Two m-tiles per unit; their half-contraction chunk-4 matmuls
            sit at row groups 0-1 / 2-3 and run concurrently."""
            mta, mtb = 2 * mp, 2 * mp + 1
            wa = slice(L * n + 128 * mta, L * n + 128 * (mta + 1))
            wb = slice(L * n + 128 * mtb, L * n + 128 * (mtb + 1))
            pa = psG.tile([128, 512], F32, tag="g")
            pb = psS.tile([128, 512], F32, tag="s")
            for kt in range(4):
                nc.tensor.matmul(pa[:, :w], attn_sb[:, kt, wa],
                                 wdT[:, kt, col:col + w],
                                 start=(kt == 0), stop=False)
            for kt in range(4):
                nc.tensor.matmul(pb[:, :w], attn_sb[:, kt, wb],
                                 wdT[:, kt, col:col + w],
                                 start=(kt == 0), stop=False)
            nc.tensor.matmul(pa[:, :w], attn_sb[0:64, 4, wa],
                             wdT[0:64, 4, col:col + w], start=False, stop=True)
            nc.tensor.matmul(pb[:, :w], attn_sb[64:128, 4, wb],
                             wdT[64:128, 4, col:col + w], start=False, stop=True)
            ota = otp.tile([128, 512], BF16, tag="ot")
            nc.scalar.activation(ota[:, :w], pa[:, :w], COPYF)
            nc.sync.dma_start(out[wa, col:col + w], ota[:, :w])
            otb = otp.tile([128, 512], BF16, tag="ot")
            nc.vector.tensor_copy(otb[:, :w], pb[:, :w])
            nc.sync.dma_start(out[wb, col:col + w], otb[:, :w])

        def dense_units(wdT, n):
            for mp in range(4):
                col = 0
                for w in CCH:
                    yield (lambda n=n, mp=mp, col=col, w=w:
                           dense_unit(wdT, n, mp, col, w))
                    col += w

        # ---------- drivers ----------
        def gen_queue_fillers(gens, n_units):
            """n_units filler thunks advancing the shared generator queue."""
            q = list(gens)
            def unit():
                while q:
                    try:
                        next(q[0])
                        return
                    except StopIteration:
                        q.pop(0)
            return [unit] * n_units

        def run_heads(n, fillers):
            """Drive head pairs (plus the solo 9th head) with 2 in-flight
            generators; consume fillers uniformly."""
            queue = [lambda h0=h0: attn_pair(n, h0, h0 + 1)
                     for h0 in range(0, HPC - 1, 2)]
            queue.append(lambda: attn_head(n, HPC - 1))
            slots = [queue.pop(0)()]   # one pair (or solo) at a time
            est_rounds = 150
            pace = len(fillers) / est_rounds
            acc = 0.0
            fi = 0
            while slots:
                for g in list(slots):
                    try:
                        next(g)
                    except StopIteration:
                        slots.remove(g)
                        if queue:
                            slots.append(queue.pop(0)())
                acc += pace
                while acc >= 1.0 and fi < len(fillers):
                    fillers[fi]()
                    fi += 1
                    acc -= 1.0
            while fi < len(fillers):
                fillers[fi]()
                fi += 1

        # stage 1: QKV+rope for batch 0, sequential
        for mc in range(CPB):
            for _ in qkv_chunk(mc):
                pass

        # batch-0 attention with batch-1 QKV as PE filler
        run_heads(0, gen_queue_fillers(
            [qkv_chunk(mc) for mc in range(CPB, NMC)], 37))

        # wq/hs dead; reuse their SBUF region for wd (WAR deps auto-inserted)
        stageA.close()
        stageB = ExitStack()
        wdp = stageB.enter_context(tc.tile_pool(name="wdp", bufs=1))
        wdT = wdp.tile([128, QPAD // 128, D], BF16)
        wd_r = wd_bf[:].rearrange("(kt p) c -> p kt c", p=128)
        wcol = 0
        for w in CCH:
            nc.sync.dma_start(wdT[:, :, wcol:wcol + w], wd_r[:, :, wcol:wcol + w])
            wcol += w

        # batch-1 attention with batch-0 dense as PE filler
        run_heads(1, list(dense_units(wdT, 0)))

        # drain: batch-1 dense
        for u in dense_units(wdT, 1):
            u()
        stageB.close()

        if _DEBUG_DUMPS:
            nc.sync.dma_start(
                dbg_qk[:].rearrange("p (c m) -> p c m", c=5), qk_bf[:])
            nc.sync.dma_start(
                dbg_attn[:].rearrange("p (c m) -> p c m", c=5), attn_sb[:])
            nc.sync.dma_start(
                dbg_vnat[:].rearrange("p (j d) -> p j d", j=N * 8), v_nat[:])
            nc.sync.dma_start(
                dbg_kd[:].rearrange("p (n l) -> p n l", n=N), kT_dup[:])

    nc.compile()
    return nc


_NC_CACHE = None


def _get_nc():
    global _NC_CACHE
    if _NC_CACHE is None:
        _NC_CACHE = _build()
    return _NC_CACHE


def _host_inputs(hidden_states, w_qkv, w_dense):
    """Build the per-core input maps (transpose + slice + bf16 cast on host)."""
    hs = np.asarray(hidden_states, dtype=np.float32).reshape(M, D)
    w_qkv = np.asarray(w_qkv, dtype=np.float32)
    w_dense = np.asarray(w_dense, dtype=np.float32)
    hs_bf = np.zeros((DP, M), dtype=ml_dtypes.bfloat16)
    hs_bf[:D, :] = np.ascontiguousarray(hs.T).astype(ml_dtypes.bfloat16)

    # RoPE tables, transposed to [dkv, l], duplicated on partitions 0-63 / 64-127
    inv_freq = 1.0 / (ROPE_BASE ** (np.arange(0, DKV, 2, dtype=np.float32) / DKV))
    t = np.arange(L, dtype=np.float32)
    freqs = np.outer(t, inv_freq)
    emb = np.concatenate([freqs, freqs], axis=-1)        # [L, DKV]
    cosT = np.cos(emb).T.astype(np.float32)              # [DKV, L]
    sinT = np.sin(emb).T.astype(np.float32)
    cos2 = np.concatenate([cosT, cosT], axis=0)
    sin2 = np.concatenate([sinT, sinT], axis=0)

    # tri[j, q] = 1 if j <= q (within-tile causal mask)
    tri = (np.arange(128)[:, None] <= np.arange(128)[None, :]).astype(
        ml_dtypes.bfloat16)

    # RoPE rotation: (P x)[d] = -x[d+32] (d<32), x[d-32] (d>=32); lhsT = P.T, 2 blocks
    P1 = np.zeros((DKV, DKV), dtype=np.float32)
    for d in range(32):
        P1[d, d + 32] = -1.0
        P1[d + 32, d] = 1.0
    PT = P1.T
    prope2 = np.zeros((128, 128), dtype=np.float32)
    prope2[:64, :64] = PT
    prope2[64:, 64:] = PT

    ident64 = np.eye(64, dtype=np.float32)

    kv_bf = w_qkv[H * DKV:, :].T.astype(ml_dtypes.bfloat16)   # [D, 128]
    in_maps = []
    for c in range(NCORES):
        h0 = HPC * c
        nh = min(HPC, H - h0)
        wq_loc = np.zeros((DP, RROWS), dtype=ml_dtypes.bfloat16)
        wq_loc[:D, :nh * DKV] = w_qkv[h0 * DKV:(h0 + nh) * DKV, :].T.astype(
            ml_dtypes.bfloat16)
        wq_loc[:D, QROWS:] = kv_bf

        # dense weight rows for this core's heads: w_dense columns
        # [64*h0 : 64*(h0+nh)) transposed, zero-padded to QPAD rows
        wd_loc = np.zeros((QPAD, D), dtype=ml_dtypes.bfloat16)
        wd_loc[:nh * DKV, :] = w_dense[:, DKV * h0:DKV * (h0 + nh)].T.astype(
            ml_dtypes.bfloat16)
        # replicate the 5th-chunk rows (head-slot 8) into the pad rows so the
        # paired dense chunk-4 matmul can read them at partitions 64-127
        wd_loc[QROWS:QPAD, :] = wd_loc[8 * DKV:QROWS, :]

        in_maps.append({
            "hs_bf": hs_bf,
            "wq_bf": wq_loc,
            "wd_bf": wd_loc,
            "cos_in": cos2,
            "sin_in": sin2,
            "tri": tri,
            "prope_in": prope2,
            "id64_in": ident64,
            "ones_in": np.ones((1, 64), dtype=np.float32),
            "colones_in": np.ones((128, 16), dtype=ml_dtypes.bfloat16),
        })
    return in_maps


def kernel(hidden_states, w_qkv, w_dense, _trace=False, _trace_kwargs=None):
    nc = _get_nc()
    in_maps = _host_inputs(hidden_states, w_qkv, w_dense)
    kw = {}
    if _trace:
        kw = dict(trace=True, **(_trace_kwargs or {}))
    res = run_bass_kernel_spmd(nc, in_maps, list(range(NCORES)), **kw)
    full = res.results[0]["out"].astype(np.float32)
    for c in range(1, NCORES):
        full += res.results[c]["out"].astype(np.float32)
    kernel._last_exec_time_ns = res.exec_time_ns
    return full.reshape(N, L, D).astype(np.float32)
